# revision 1
# baseline (speedup 1.0000x reference)
"""Trainium2 Bass kernel for a recurrent adaptive-LIF SNN.

Network (per reference):
    B=1024, T=100, n_in=120, h1=512, h2=256, n_out=35
    per step t:
        cur1 = x_t @ W1.T + s1 @ Wrec.T
        a1' = rho1*a1 + (1-rho1)*s1
        v1' = alpha1*v1*(1-s1) + (1-alpha1)*cur1
        s1' = (v1' - (1 + beta_a1*a1') > 0)
        cur2 = s1' @ W2.T ; same LIF for layer 2
        vo' = beta_out*vo + (1-beta_out)*(s2' @ W3.T)
    out = mean_t vo(t)

Sharding: data-parallel over batch across 8 cores (128 batch/core),
weights replicated; the sequential T loop is local per core.

Layout: feature-major — [feature -> partitions, batch -> free].  The
recurrent matmuls then consume spike states directly as the moving
operand (no transposes) and per-feature constants (all uniform) become
immediates.

Layer-1 reformulation (exact algebra; P1 := v1' - 1 lives in PSUM):
    u1' = rho1*u1 + s1           (u1 := beta_a1*a1 / cb1, cb1 = beta_a1*(1-rho1))
    s1' = (cb1*u1' < P1)         (== v1' - (1+beta_a1*a1') > 0)
    q1' = (s1'-1)*P1             (== -(v1'-1)*(1-s1'))
    P1  = W1aug@[x;1] + WrecF@s1 + (-a1*I)@q1
      where W1aug has an extra row alpha1-1 against a constant-1 input row,
      WrecF = ((1-alpha1)Wrec).T - alpha1*I, and q1 init = 1 (since v=0).
Layer 2 (unshifted; P2 := v2'):
    t2 = cb2*s2 (ACT);  w2' = rho2*w2 + t2;  s2' = (w2'+1 < P2)
    q2' = (s2'-1)*P2;   P2 = W2s@s1 + (-a2*I)@q2,  q2 init = 0
Output:
    vo_psum = (b*I)@vo + W3s@s2 ; vo = ACT copy;  SUM += I@vo in PSUM
with (1-alpha)/(1-beta_out) folded into W1/Wrec/W2/W3 on the host.
"""

import sys
import numpy as np

sys.path.insert(0, "/opt/trn_rl_repo")

import ml_dtypes

bf16 = ml_dtypes.bfloat16

# Problem constants (hardcoded per contract)
B, T, N_IN, H1, H2, N_OUT = 1024, 100, 120, 512, 256, 35
N_CORES = 8
BC = B // N_CORES  # 128 batch per core
C1 = H1 // 128     # 4 feature chunks, layer 1
C2 = H2 // 128     # 2 feature chunks, layer 2
K1 = N_IN + 1      # x augmented with a constant-one row

_CACHE = {}


def _build(alpha1, rho1, beta_a1, alpha2, rho2, beta_a2, beta_out):
    import concourse.bacc as bacc
    import concourse.mybir as mybir
    import concourse.tile as tile
    from concourse.alu_op_type import AluOpType

    fp32 = mybir.dt.float32
    bft = mybir.dt.bfloat16
    A = AluOpType
    IDENT = mybir.ActivationFunctionType.Identity

    cb1 = float(beta_a1 * (1.0 - rho1))
    cb2 = float(beta_a2 * (1.0 - rho2))

    nc = bacc.Bacc()

    x_d = nc.declare_dram_parameter("x", [K1, T, BC], bft, isOutput=False)
    w1_d = nc.declare_dram_parameter("w1s", [K1, C1, 128], bft, isOutput=False)
    wr_d = nc.declare_dram_parameter("wrecs", [128, C1, C1, 128], bft, isOutput=False)
    w2_d = nc.declare_dram_parameter("w2s", [128, C1, C2, 128], bft, isOutput=False)
    w3_d = nc.declare_dram_parameter("w3s", [128, C2, N_OUT], bft, isOutput=False)
    ai1_d = nc.declare_dram_parameter("negai1", [128, 128], bft, isOutput=False)
    ai2_d = nc.declare_dram_parameter("negai2", [128, 128], bft, isOutput=False)
    i35_d = nc.declare_dram_parameter("i35", [N_OUT, 2, N_OUT], bft, isOutput=False)
    out_d = nc.declare_dram_parameter("out", [N_OUT, BC], fp32, isOutput=True)

    XCH = 10  # x preload chunks
    TP = T // XCH

    with tile.TileContext(nc) as tc:
        with (
            tc.tile_pool(name="wpool", bufs=1) as wpool,
            tc.tile_pool(name="xpool", bufs=1) as xpool,
            tc.tile_pool(name="st1", bufs=3) as st1,
            tc.tile_pool(name="st2", bufs=3) as st2,
            tc.tile_pool(name="tmp", bufs=3) as tmp,
            tc.tile_pool(name="ps1", bufs=3, space="PSUM") as ps1,
            tc.tile_pool(name="ps2", bufs=2, space="PSUM") as ps2,
            tc.tile_pool(name="pso", bufs=2, space="PSUM") as pso,
            tc.tile_pool(name="pssum", bufs=1, space="PSUM") as pssum,
        ):
            # ---- resident weights ----
            w1_s = wpool.tile([K1, C1, 128], bft, tag="w1")
            nc.sync.dma_start(w1_s[:], w1_d[:])
            wr_s = wpool.tile([128, C1, C1, 128], bft, tag="wr")
            nc.sync.dma_start(wr_s[:], wr_d[:])
            w2_s = wpool.tile([128, C1, C2, 128], bft, tag="w2")
            nc.sync.dma_start(w2_s[:], w2_d[:])
            w3_s = wpool.tile([128, C2, N_OUT], bft, tag="w3")
            nc.sync.dma_start(w3_s[:], w3_d[:])
            ai1_s = wpool.tile([128, 128], bft, tag="ai1")
            nc.sync.dma_start(ai1_s[:], ai1_d[:])
            ai2_s = wpool.tile([128, 128], bft, tag="ai2")
            nc.sync.dma_start(ai2_s[:], ai2_d[:])
            # i35[:,0,:] = identity, i35[:,1,:] = beta_out * identity
            i35_s = wpool.tile([N_OUT, 2, N_OUT], bft, tag="i35")
            nc.sync.dma_start(i35_s[:], i35_d[:])

            # ---- x preload in chunks ----
            x_tiles = []
            for i in range(XCH):
                xt = xpool.tile([K1, TP, BC], bft, tag=f"x{i}")
                nc.sync.dma_start(xt[:], x_d[:, i * TP : (i + 1) * TP, :])
                x_tiles.append(xt)

            # ---- initial states ----
            s1 = st1.tile([128, C1 * BC], bft, tag="s1")
            q1 = st1.tile([128, C1 * BC], bft, tag="q1")
            u1 = st1.tile([128, C1 * BC], bft, tag="u1")
            s2 = st2.tile([128, C2 * BC], bft, tag="s2")
            q2 = st2.tile([128, C2 * BC], bft, tag="q2")
            w2st = st2.tile([128, C2 * BC], bft, tag="w2st")
            for z, val in ((s1, 0.0), (q1, 1.0), (u1, 0.0),
                           (s2, 0.0), (q2, 0.0), (w2st, 0.0)):
                nc.vector.memset(z[:], val)
            vo = tmp.tile([N_OUT, BC], bft, tag="vo")
            nc.vector.memset(vo[:], 0.0)

            sum_ps = pssum.tile([N_OUT, BC], fp32, tag="sum")

            for t in range(T):
                xsl = x_tiles[t // TP][:, t % TP, :]

                # ----- P1 = v1' - 1 -----
                p1 = ps1.tile([128, C1 * BC], fp32, tag="p1")
                for m in range(C1):
                    o = p1[:, m * BC : (m + 1) * BC]
                    nc.tensor.matmul(o, w1_s[:, m, :], xsl, start=True, stop=False)
                    for k in range(C1):
                        nc.tensor.matmul(
                            o, wr_s[:, k, m, :], s1[:, k * BC : (k + 1) * BC],
                            start=False, stop=False,
                        )
                    nc.tensor.matmul(
                        o, ai1_s[:], q1[:, m * BC : (m + 1) * BC],
                        start=False, stop=True,
                    )

                # ----- layer 1 state update -----
                u1n = st1.tile([128, C1 * BC], bft, tag="u1")
                nc.vector.scalar_tensor_tensor(
                    u1n[:], u1[:], float(rho1), s1[:], A.mult, A.add
                )
                s1n = st1.tile([128, C1 * BC], bft, tag="s1")
                nc.vector.scalar_tensor_tensor(
                    s1n[:], u1n[:], cb1, p1[:], A.mult, A.is_lt
                )
                q1n = st1.tile([128, C1 * BC], bft, tag="q1")
                nc.vector.scalar_tensor_tensor(
                    q1n[:], s1n[:], 1.0, p1[:], A.subtract, A.mult
                )
                s1, q1, u1 = s1n, q1n, u1n

                # ----- P2 = v2' -----
                p2 = ps2.tile([128, C2 * BC], fp32, tag="p2")
                for m in range(C2):
                    o = p2[:, m * BC : (m + 1) * BC]
                    for k in range(C1):
                        nc.tensor.matmul(
                            o, w2_s[:, k, m, :], s1[:, k * BC : (k + 1) * BC],
                            start=(k == 0), stop=False,
                        )
                    nc.tensor.matmul(
                        o, ai2_s[:], q2[:, m * BC : (m + 1) * BC],
                        start=False, stop=True,
                    )

                # ----- layer 2 state update -----
                p2b = tmp.tile([128, C2 * BC], bft, tag="p2b")
                nc.scalar.activation(p2b[:], p2[:], IDENT)
                t2 = tmp.tile([128, C2 * BC], bft, tag="t2")
                nc.scalar.activation(t2[:], s2[:], IDENT, scale=cb2)
                w2n = st2.tile([128, C2 * BC], bft, tag="w2st")
                nc.vector.scalar_tensor_tensor(
                    w2n[:], w2st[:], float(rho2), t2[:], A.mult, A.add
                )
                s2n = st2.tile([128, C2 * BC], bft, tag="s2")
                nc.vector.scalar_tensor_tensor(
                    s2n[:], w2n[:], 1.0, p2b[:], A.add, A.is_lt
                )
                q2n = st2.tile([128, C2 * BC], bft, tag="q2")
                nc.vector.scalar_tensor_tensor(
                    q2n[:], s2n[:], 1.0, p2b[:], A.subtract, A.mult
                )
                s2, q2, w2st = s2n, q2n, w2n

                # ----- output integrator on PE -----
                yp = pso.tile([N_OUT, BC], fp32, tag="y")
                nc.tensor.matmul(yp[:], i35_s[:, 1, :], vo[:], start=True, stop=False)
                for k in range(C2):
                    nc.tensor.matmul(
                        yp[:], w3_s[:, k, :], s2[:, k * BC : (k + 1) * BC],
                        start=False, stop=(k == C2 - 1),
                    )
                von = tmp.tile([N_OUT, BC], bft, tag="vo")
                nc.scalar.activation(von[:], yp[:], IDENT)
                vo = von

                nc.tensor.matmul(
                    sum_ps[:], i35_s[:, 0, :], vo[:],
                    start=(t == 0), stop=(t == T - 1),
                    skip_group_check=True,
                )

            outf = tmp.tile([N_OUT, BC], fp32, tag="outf")
            nc.vector.tensor_scalar(outf[:], sum_ps[:], 1.0 / T, None, A.mult)
            nc.sync.dma_start(out_d[:], outf[:])

    nc.compile()
    return nc


def _prep_inputs(x, W1, Wrec, W2, W3, alpha1, rho1, beta_a1, alpha2, rho2, beta_a2, beta_out):
    a1 = float(np.asarray(alpha1).reshape(-1)[0])
    a2 = float(np.asarray(alpha2).reshape(-1)[0])
    bo = float(np.asarray(beta_out).reshape(-1)[0])

    w1s = ((1.0 - np.asarray(alpha1, np.float32)[:, None]) * np.asarray(W1, np.float32)).T
    wrs = ((1.0 - np.asarray(alpha1, np.float32)[:, None]) * np.asarray(Wrec, np.float32)).T
    w2s = ((1.0 - np.asarray(alpha2, np.float32)[:, None]) * np.asarray(W2, np.float32)).T
    w3s = ((1.0 - np.asarray(beta_out, np.float32)[:, None]) * np.asarray(W3, np.float32)).T

    # layer-1 shift folds:  WrecF = wrs - a1*I ; W1 gains const row (a1-1)
    wrs = wrs - a1 * np.eye(H1, dtype=np.float32)
    w1aug = np.concatenate(
        [w1s, np.full((1, H1), a1 - 1.0, np.float32)], axis=0
    )  # [121, 512]

    w1_a = np.ascontiguousarray(w1aug.reshape(K1, C1, 128)).astype(bf16)
    wr_a = np.ascontiguousarray(
        wrs.reshape(C1, 128, C1, 128).transpose(1, 0, 2, 3)
    ).astype(bf16)
    w2_a = np.ascontiguousarray(
        w2s.reshape(C1, 128, C2, 128).transpose(1, 0, 2, 3)
    ).astype(bf16)
    w3_a = np.ascontiguousarray(
        w3s.reshape(C2, 128, N_OUT).transpose(1, 0, 2)
    ).astype(bf16)

    nai1 = (-a1 * np.eye(128, dtype=np.float32)).astype(bf16)
    nai2 = (-a2 * np.eye(128, dtype=np.float32)).astype(bf16)
    i35 = np.stack(
        [np.eye(N_OUT, dtype=np.float32), bo * np.eye(N_OUT, dtype=np.float32)], axis=1
    ).astype(bf16)  # [35, 2, 35]

    shared = dict(
        w1s=w1_a, wrecs=wr_a, w2s=w2_a, w3s=w3_a,
        negai1=nai1, negai2=nai2, i35=i35,
    )
    in_maps = []
    for c in range(N_CORES):
        xc = np.asarray(x[c * BC : (c + 1) * BC], np.float32)  # [BC, T, N_IN]
        xfm = xc.transpose(2, 1, 0)  # [N_IN, T, BC]
        xaug = np.concatenate([xfm, np.ones((1, T, BC), np.float32)], axis=0)
        in_maps.append(dict(x=np.ascontiguousarray(xaug).astype(bf16), **shared))
    return in_maps


def kernel(
    x, W1, Wrec, W2, W3,
    alpha1, rho1, beta_a1, alpha2, rho2, beta_a2, beta_out,
    _trace=False,
):
    from concourse.bass_utils import run_bass_kernel_spmd

    key = "nc"
    if key not in _CACHE:
        _CACHE[key] = _build(
            float(np.asarray(alpha1).reshape(-1)[0]),
            float(np.asarray(rho1).reshape(-1)[0]),
            float(np.asarray(beta_a1).reshape(-1)[0]),
            float(np.asarray(alpha2).reshape(-1)[0]),
            float(np.asarray(rho2).reshape(-1)[0]),
            float(np.asarray(beta_a2).reshape(-1)[0]),
            float(np.asarray(beta_out).reshape(-1)[0]),
        )
    nc = _CACHE[key]

    in_maps = _prep_inputs(
        x, W1, Wrec, W2, W3, alpha1, rho1, beta_a1, alpha2, rho2, beta_a2, beta_out
    )
    res = run_bass_kernel_spmd(nc, in_maps, list(range(N_CORES)), trace=_trace)

    out = np.empty((B, N_OUT), np.float32)
    for c in range(N_CORES):
        out[c * BC : (c + 1) * BC] = np.asarray(res.results[c]["out"]).T
    if _trace:
        return out, res
    return out

